# revision 3
# baseline (speedup 1.0000x reference)
"""Trainium2 Bass kernel v2 for causal multi-head attention with RoPE.

B=4, S=2048, D=1024, H=16, DK=64; fp32 in/out.
Sharding: core c = (batch c//2, head-group c%2 of 8 heads); host sums the
two partial o_proj outputs per batch.

Design vs original baseline (733us):
  - Single interleaved pipeline: iteration sc projects s-chunk sc (q,v,k)
    while the attention row block qg=sc-1 (scores->exp->attnv->normalize->
    o_proj) is round-robined into the same PE instruction stream, so the
    softmax exp (ACT) and rope/normalize (DVE) overlap projection matmuls
    instead of being phase-separated.
  - bf16 for all streamed operands (x, Wq/Wk/Wv, q/k/v, attention probs,
    Wo, cos/sin); PSUM accumulation and softmax normalization stay fp32.
    Measured rel err ~3.7e-3 vs the 2e-2 gate.
  - exp batched over both tile-packed heads: one activation instruction
    reads a [128,1024] PSUM pair (2 banks).
  - Causal mask applied post-exp as a non-aliased 0/1 bf16 multiply on the
    otherwise-idle Pool engine (diagonal attnv split into masked/unmasked
    column ranges).
  - Normalized attention outputs written directly into the packed o_proj
    lhsT tile via partition-shifted DVE writes (no SBUF-to-SBUF DMA).
  - PSUM: one shared [128,1024]x2 pool for projection/score pairs and one
    [128,512]x4 pool for attnv accumulators + denominator broadcast +
    o_proj tiles (exactly 8 banks).
  - Batched descriptor-strided DMAs (one per weight matrix / x chunk /
    output half); first-consumer-ordered startup loads.
"""

import sys

sys.path.insert(0, "/opt/trn_rl_repo")

from contextlib import ExitStack

import numpy as np
import ml_dtypes

import concourse.bass as bass
import concourse.tile as tile
from concourse import bacc, mybir
from concourse.bass_utils import run_bass_kernel_spmd

B, S, D, H = 4, 2048, 1024, 16
DK = D // H          # 64
NHL = 8              # heads per core
QR = NHL * DK        # 512 projected rows per core
NKC = S // 128       # 16 kv chunks
NSC = 4              # s-chunks of 512
THETA = 10000.0

F32 = mybir.dt.float32
F32R = mybir.dt.float32r
BF16 = mybir.dt.bfloat16
NP_BF16 = ml_dtypes.bfloat16

_COMPILED = None


def _r(ap):
    return ap.bitcast(F32R)


def build_kernel():
    nc = bacc.Bacc("TRN2", target_bir_lowering=False, debug=False,
                   enable_asserts=False)

    xT = nc.dram_tensor("xT", [D, S], BF16, kind="ExternalInput").ap()
    wqT = nc.dram_tensor("wqT", [D, QR], BF16, kind="ExternalInput").ap()
    wkT = nc.dram_tensor("wkT", [D, QR], BF16, kind="ExternalInput").ap()
    wvT = nc.dram_tensor("wvT", [D, QR], BF16, kind="ExternalInput").ap()
    woTb = nc.dram_tensor("woTb", [QR, D], BF16, kind="ExternalInput").ap()
    cosd = nc.dram_tensor("cosd", [128, S], BF16, kind="ExternalInput").ap()
    sind = nc.dram_tensor("sind", [128, S], BF16, kind="ExternalInput").ap()
    maskd = nc.dram_tensor("maskd", [128, 128], BF16, kind="ExternalInput").ap()
    onesd = nc.dram_tensor("onesd", [1, 64], F32, kind="ExternalInput").ap()
    vinit = nc.dram_tensor("vinit", [128, NKC * NHL * 65], BF16,
                           kind="ExternalInput").ap()
    out = nc.dram_tensor("out", [S, D], F32, kind="ExternalOutput").ap()

    Exp = mybir.ActivationFunctionType.Exp

    with tile.TileContext(nc) as tc, ExitStack() as ctx:
        persist = ctx.enter_context(tc.tile_pool(name="persist", bufs=1))

        # ---- persistent SBUF ----
        wq = persist.tile([128, 8 * QR], BF16, tag="wq")
        wk = persist.tile([128, 8 * QR], BF16, tag="wk")
        wv = persist.tile([128, 8 * QR], BF16, tag="wv")
        wo = persist.tile([128, 4 * D], BF16, tag="wo")
        cosall = persist.tile([128, S], BF16, tag="cosall")
        sinall = persist.tile([128, S], BF16, tag="sinall")
        qall = persist.tile([128, 4 * S], BF16, tag="qall")
        kall = persist.tile([128, 4 * S], BF16, tag="kall")
        v_all = persist.tile([128, NKC * NHL * 65], BF16, tag="v_all")
        maskt = persist.tile([128, 128], BF16, tag="maskt")
        onest = persist.tile([1, 64], F32R, tag="onest")
        osb = persist.tile([128, 8 * 512], F32, tag="osb")
        osb = persist.tile([128, 8 * 512], F32, tag="osb")

        # weight loads: wq first (first consumer), rest behind it; split
        # across trigger queues so HWDGE overhead overlaps
        def _wload(wsb, wdr, eng):
            eng.dma_start(
                wsb[:].rearrange("p (kk c) -> p kk c", kk=8),
                wdr[:].rearrange("(kk p) c -> p kk c", kk=8))
        _wload(wq, wqT, nc.sync)

        # ---- pools ----
        xpool = ctx.enter_context(tc.tile_pool(name="xp", bufs=2))
        bigp = ctx.enter_context(tc.tile_pool(name="bigp", bufs=2,
                                              space="PSUM"))
        opool = ctx.enter_context(tc.tile_pool(name="op", bufs=4,
                                               space="PSUM"))
        qspool = ctx.enter_context(tc.tile_pool(name="qsp", bufs=4))
        rtpool = ctx.enter_context(tc.tile_pool(name="rtp", bufs=4))
        stgpool = ctx.enter_context(tc.tile_pool(name="stp", bufs=6))
        ptpool = ctx.enter_context(tc.tile_pool(name="ptp", bufs=4))
        pairpool = ctx.enter_context(tc.tile_pool(name="prp", bufs=8))
        smallp = ctx.enter_context(tc.tile_pool(name="smp", bufs=4))
        rlpool = ctx.enter_context(tc.tile_pool(name="rlp", bufs=2))

        state = {}

        # ---------- unit builders (each returns a closure to emit) ----------
        def late_loads():
            _wload(wv, wvT, nc.scalar)
            _wload(wk, wkT, nc.sync)
            nc.scalar.dma_start(
                wo[:].rearrange("p (pp c) -> p pp c", pp=4),
                woTb[:].rearrange("(pp p) c -> p pp c", pp=4))
            nc.sync.dma_start(cosall[:], cosd[:])
            nc.sync.dma_start(sinall[:], sind[:])
            nc.scalar.dma_start(maskt[:], maskd[:])
            nc.scalar.dma_start(onest[:], _r(onesd[:]))
            nc.scalar.dma_start(v_all[:], vinit[:])

        def xload(sc):
            def emit():
                xt = xpool.tile([128, 8 * 512], BF16, tag="xt", name="xt")
                nc.sync.dma_start(
                    xt[:].rearrange("p (kk c) -> p kk c", kk=8),
                    xT[:, sc * 512:(sc + 1) * 512]
                    .rearrange("(kk p) c -> p kk c", kk=8))
                state[("xt", sc)] = xt
            return emit

        def proj_mm(sc, which, mp, i):
            # q/k half m-pair projection (one m-group of 8 MMs)
            def emit():
                xt = state[("xt", sc)]
                wt = wq if which == "q" else wk
                if i == 0:
                    state[(which, sc, mp)] = bigp.tile(
                        [128, 1024], F32, tag="big", name=f"ps{which}")
                ps = state[(which, sc, mp)]
                m = 2 * mp + i
                for kk in range(8):
                    nc.tensor.matmul(
                        ps[:, 512 * i:512 * (i + 1)],
                        wt[:, kk * QR + m * 128: kk * QR + (m + 1) * 128],
                        xt[:, kk * 512:(kk + 1) * 512],
                        start=(kk == 0), stop=(kk == 7))
            return emit

        def proj_cp(sc, which, mp):
            def emit():
                ps = state[(which, sc, mp)]
                qs = qspool.tile([128, 1024], BF16, tag="qs", name="qs")
                nc.vector.tensor_copy(qs[:], ps[:])
                state[(which + "s", sc, mp)] = qs
            return emit

        def v_mm(sc, stp, i):
            def emit():
                xt = state[("xt", sc)]
                if i == 0:
                    state[("v", sc, stp)] = bigp.tile(
                        [128, 1024], F32, tag="big", name="psv")
                ps = state[("v", sc, stp)]
                st = 2 * stp + i
                for kk in range(8):
                    nc.tensor.matmul(
                        ps[:, 512 * i:512 * (i + 1)],
                        xt[:, kk * 512 + st * 128: kk * 512 + (st + 1) * 128],
                        wv[:, kk * QR:(kk + 1) * QR],
                        start=(kk == 0), stop=(kk == 7))
            return emit

        def v_cp(sc, stp):
            def emit():
                ps = state[("v", sc, stp)]
                ck0 = sc * 4 + 2 * stp
                dst = v_all[:, ck0 * NHL * 65:(ck0 + 2) * NHL * 65].rearrange(
                    "p (g c) -> p g c", c=65)[:, :, 0:64]
                src = ps[:].rearrange("p (g c) -> p g c", c=64)
                nc.scalar.copy(dst, src)
            return emit

        def rope(sc, which):
            # consumes qs pairs (mp=0: chunks he03|he47, mp=1: ho03|ho47)
            def emit():
                s0 = sc * 512
                A = state[(which + "s", sc, 0)]
                Bt = state[(which + "s", sc, 1)]
                dst = qall if which == "q" else kall
                cosc = cosall[:, s0:s0 + 512]
                sinc = sinall[:, s0:s0 + 512]
                for pi, hbase in ((0, 0), (1, 4)):
                    e_in = A[:, 512 * pi:512 * (pi + 1)]
                    o_in = Bt[:, 512 * pi:512 * (pi + 1)]
                    te = rtpool.tile([128, 512], BF16, tag="rt", name="te")
                    to = rtpool.tile([128, 512], BF16, tag="rt", name="to")
                    nc.vector.tensor_mul(te[:], e_in, cosc)
                    nc.vector.tensor_mul(to[:], o_in, sinc)
                    qre = stgpool.tile([128, 512], BF16, tag="st", name="qre")
                    nc.vector.tensor_sub(qre[:], te[:], to[:])
                    te2 = rtpool.tile([128, 512], BF16, tag="rt", name="te2")
                    to2 = rtpool.tile([128, 512], BF16, tag="rt", name="to2")
                    nc.vector.tensor_mul(te2[:], o_in, cosc)
                    nc.vector.tensor_mul(to2[:], e_in, sinc)
                    qro = stgpool.tile([128, 512], BF16, tag="st", name="qro")
                    nc.vector.tensor_add(qro[:], te2[:], to2[:])
                    # permute into head-contiguous layout: head h (chunk h//2)
                    # -> partitions 64*(h%2) + 32*eo + j
                    for src_t, eo in ((qre, 0), (qro, 1)):
                        for hl in range(4):
                            h = hbase + hl
                            hp, h01 = h // 2, h % 2
                            p0 = 64 * h01 + 32 * eo
                            eng = nc.sync if eo == 0 else nc.gpsimd
                            eng.dma_start(
                                dst[p0:p0 + 32, hp * S + s0:hp * S + s0 + 512],
                                src_t[32 * hl:32 * hl + 32, :])
            return emit

        # ---------- attention units ----------
        def attn_pass(qg, hp, c, nchunks):
            def emit():
                q0 = qg * 512
                cmod = c - 4 * qg
                qoff = 128 * cmod if cmod >= 0 else 0
                N = 512 - qoff
                sp = bigp.tile([128, 1024], F32, tag="big", name="sp")
                for h01 in range(2):
                    base = 64 * h01
                    nc.tensor.matmul(
                        sp[:, 512 * h01:512 * h01 + N],
                        kall[base:base + 64, hp * S + c * 128:hp * S + (c + 1) * 128],
                        qall[base:base + 64,
                             hp * S + q0 + qoff:hp * S + q0 + qoff + N],
                        start=True, stop=True, tile_position=(base, 0))
                pt = ptpool.tile([128, 1024], BF16, tag="pt", name="pt")
                if N == 512:
                    nc.scalar.activation(pt[:], sp[:], Exp, scale=0.125)
                else:
                    for h01 in range(2):
                        nc.scalar.activation(
                            pt[:, 512 * h01:512 * h01 + N],
                            sp[:, 512 * h01:512 * h01 + N], Exp, scale=0.125)
                if cmod >= 0:
                    # multiplicative 0/1 causal mask on the diagonal block,
                    # non-aliased bf16 on Pool; attnv reads the masked copy
                    ptd = ptpool.tile([128, 256], BF16, tag="ptd", name="ptd",
                                      bufs=4)
                    for h01 in range(2):
                        nc.gpsimd.tensor_mul(
                            ptd[:, 128 * h01:128 * h01 + 128],
                            pt[:, 512 * h01:512 * h01 + 128], maskt[:])
                if c == 0:
                    state[("O", hp, 0)] = opool.tile([128, 512], F32, tag="O",
                                                     name="O0")
                    state[("O", hp, 1)] = opool.tile([128, 512], F32, tag="O",
                                                     name="O1")
                for h01 in range(2):
                    O = state[("O", hp, h01)]
                    vsl = v_all[:, (c * NHL + 2 * hp + h01) * 65:
                                (c * NHL + 2 * hp + h01) * 65 + 65]
                    if cmod >= 0:
                        nc.tensor.matmul(
                            O[0:65, qoff:qoff + 128], vsl,
                            ptd[:, 128 * h01:128 * h01 + 128],
                            start=(c == 0), stop=(c == nchunks - 1))
                        if N > 128:
                            nc.tensor.matmul(
                                O[0:65, qoff + 128:qoff + N], vsl,
                                pt[:, 512 * h01 + 128:512 * h01 + N],
                                start=(c == 0), stop=(c == nchunks - 1),
                                skip_group_check=True)
                    else:
                        nc.tensor.matmul(
                            O[0:65, qoff:qoff + N], vsl,
                            pt[:, 512 * h01:512 * h01 + N],
                            start=(c == 0), stop=(c == nchunks - 1))
            return emit

        def attn_norm(qg, hp):
            def emit():
                pair = pairpool.tile([128, 512], BF16, tag="pair", name="pair")
                for h01 in range(2):
                    O = state[("O", hp, h01)]
                    lsb = smallp.tile([1, 512], F32R, tag="ls", name="lsb")
                    nc.scalar.copy(lsb[:], O[64:65, :])
                    rbp = opool.tile([128, 512], F32, tag="O",
                                     name="rbp")
                    nc.tensor.matmul(rbp[0:64, :], onest[:], lsb[:],
                                     start=True, stop=True)
                    rlb = rlpool.tile([64, 512], F32, tag="rl", name="rlb")
                    nc.vector.reciprocal(rlb[:], rbp[0:64, :])
                    nc.vector.tensor_mul(pair[64 * h01:64 * h01 + 64, :],
                                         O[0:64, :], rlb[:])
                state[("pair", hp)] = pair
            return emit

        def oproj(qg, qt, oh):
            def emit():
                f = opool.tile([128, 512], F32, tag="O", name="f")
                for p in range(4):
                    nc.tensor.matmul(
                        f[:],
                        state[("pair", p)][:, qt * 128:(qt + 1) * 128],
                        wo[:, p * D + oh * 512:p * D + oh * 512 + 512],
                        start=(p == 0), stop=(p == 3))
                nc.vector.tensor_copy(
                    osb[:, (oh * 4 + qt) * 512:(oh * 4 + qt + 1) * 512],
                    f[:])
            return emit

        def store(qg, oh):
            def emit():
                src = osb[:, oh * 2048:(oh + 1) * 2048].rearrange(
                    "p (qt c) -> p qt c", qt=4)
                dst = out[qg * 512:(qg + 1) * 512,
                          oh * 512:(oh + 1) * 512].rearrange(
                    "(qt p) c -> p qt c", qt=4)
                nc.sync.dma_start(dst, src)
            return emit

        def attn_units(qg):
            units = []
            nchunks = 4 * qg + 4
            for hp in range(4):
                for c in range(nchunks):
                    units.append(attn_pass(qg, hp, c, nchunks))
                units.append(attn_norm(qg, hp))
            for oh in range(2):
                for qt in range(4):
                    units.append(oproj(qg, qt, oh))
                units.append(store(qg, oh))
            return units

        # ---------- emission: interleave proj(sc) with attn(qg=sc-1) ----------
        for sc in range(NSC):
            xload(sc)()
            if sc == 0:
                late_loads()
            proj_units = []
            for mp in range(2):
                proj_units.append(proj_mm(sc, "q", mp, 0))
                proj_units.append(proj_mm(sc, "q", mp, 1))
                proj_units.append(proj_cp(sc, "q", mp))
            for stp in range(2):
                proj_units.append(v_mm(sc, stp, 0))
                proj_units.append(v_mm(sc, stp, 1))
                proj_units.append(v_cp(sc, stp))
            for mp in range(2):
                proj_units.append(proj_mm(sc, "k", mp, 0))
                proj_units.append(proj_mm(sc, "k", mp, 1))
                proj_units.append(proj_cp(sc, "k", mp))

            at_units = attn_units(sc - 1) if sc > 0 else []
            # round-robin: spread attn units across proj units
            n_p, n_a = len(proj_units), len(at_units)
            ai = 0
            for i, pu in enumerate(proj_units):
                pu()
                take = (n_a * (i + 1)) // n_p - (n_a * i) // n_p
                for _ in range(take):
                    at_units[ai]()
                    ai += 1
            while ai < n_a:
                at_units[ai]()
                ai += 1

            rope(sc, "q")()
            rope(sc, "k")()

        for u in attn_units(NSC - 1):
            u()

    nc.compile()
    return nc


def _rope_perm():
    """Row permutation for Wq/Wk per-core slices: 4 chunks of 128 =
    (heads 0-3 even, heads 4-7 even, heads 0-3 odd, heads 4-7 odd)."""
    perm = []
    for half in (0, 1):
        for hblk in range(2):
            for h in range(4 * hblk, 4 * hblk + 4):
                for j in range(32):
                    perm.append(h * 64 + 2 * j + half)
    return np.array(perm)


def _prep_in_maps(x, token_positions, Wq, Wk, Wv, Wo):
    half = DK // 2
    freqs = (1.0 / (THETA ** (2.0 * np.arange(half, dtype=np.float32) / DK)))
    angles = token_positions.astype(np.float32)[:, None] * freqs[None, :]
    cos = np.cos(angles).astype(np.float32).T    # [32, S]
    sin = np.sin(angles).astype(np.float32).T
    cos4 = np.ascontiguousarray(np.tile(cos, (4, 1))).astype(NP_BF16)
    sin4 = np.ascontiguousarray(np.tile(sin, (4, 1))).astype(NP_BF16)

    onesd = np.ones((1, 64), dtype=np.float32)
    vinit = np.zeros((128, NKC * NHL * 65), dtype=np.float32)
    vinit.reshape(128, NKC * NHL, 65)[:, :, 64] = 1.0
    vinit = vinit.astype(NP_BF16)
    kv_l = np.arange(128)[:, None]
    q_l = np.arange(128)[None, :]
    maskd = np.where(q_l >= kv_l, 1.0, 0.0).astype(NP_BF16)

    perm = _rope_perm()
    in_maps = []
    for c in range(8):
        b, g = c // 2, c % 2
        rows = slice(g * QR, (g + 1) * QR)
        wq_g = Wq[rows, :][perm, :]
        wk_g = Wk[rows, :][perm, :]
        wv_g = Wv[rows, :]
        in_maps.append({
            "xT": np.ascontiguousarray(x[b].T).astype(NP_BF16),
            "wqT": np.ascontiguousarray(wq_g.T).astype(NP_BF16),
            "wkT": np.ascontiguousarray(wk_g.T).astype(NP_BF16),
            "wvT": np.ascontiguousarray(wv_g.T).astype(NP_BF16),
            "woTb": np.ascontiguousarray(Wo[:, rows].T).astype(NP_BF16),
            "cosd": cos4,
            "sind": sin4,
            "maskd": maskd,
            "onesd": onesd,
            "vinit": vinit,
        })
    return in_maps


def kernel(x, token_positions, Wq, Wk, Wv, Wo):
    global _COMPILED
    x = np.asarray(x, dtype=np.float32)
    token_positions = np.asarray(token_positions)
    Wq = np.asarray(Wq, dtype=np.float32)
    Wk = np.asarray(Wk, dtype=np.float32)
    Wv = np.asarray(Wv, dtype=np.float32)
    Wo = np.asarray(Wo, dtype=np.float32)

    if _COMPILED is None:
        _COMPILED = build_kernel()
    nc = _COMPILED

    in_maps = _prep_in_maps(x, token_positions, Wq, Wk, Wv, Wo)
    res = run_bass_kernel_spmd(nc, in_maps, core_ids=list(range(8)))

    out = np.empty((B, S, D), dtype=np.float32)
    for b in range(B):
        out[b] = res.results[2 * b]["out"] + res.results[2 * b + 1]["out"]
    return out


def time_device(inputs, n1=32, n2=128, repeats=2):
    """Async-pipelined device timing (see baseline kernel for details)."""
    import time

    import jax
    from jax.sharding import Mesh, NamedSharding, PartitionSpec

    try:
        from jax.experimental.shard_map import shard_map
    except ImportError:
        from jax import shard_map

    from concourse import bass2jax

    global _COMPILED
    if _COMPILED is None:
        _COMPILED = build_kernel()
    nc = _COMPILED
    bass2jax.install_neuronx_cc_hook()

    in_maps = _prep_in_maps(
        np.asarray(inputs["x"], np.float32), np.asarray(inputs["token_positions"]),
        np.asarray(inputs["Wq"], np.float32), np.asarray(inputs["Wk"], np.float32),
        np.asarray(inputs["Wv"], np.float32), np.asarray(inputs["Wo"], np.float32))

    partition_name = (nc.partition_id_tensor.name
                      if nc.partition_id_tensor else None)
    in_names, out_names, out_avals, zero_outs = [], [], [], []
    for alloc in nc.m.functions[0].allocations:
        if not isinstance(alloc, mybir.MemoryLocationSet):
            continue
        name = alloc.memorylocations[0].name
        if alloc.kind == "ExternalInput":
            if name != partition_name:
                in_names.append(name)
        elif alloc.kind == "ExternalOutput":
            out_names.append(name)
            shape = tuple(alloc.tensor_shape)
            dtype = mybir.dt.np(alloc.dtype)
            out_avals.append(jax.core.ShapedArray(shape, dtype))
            zero_outs.append(np.zeros(shape, dtype))
    n_params = len(in_names)
    all_in_names = in_names + out_names
    if partition_name is not None:
        all_in_names = all_in_names + [partition_name]

    def _body(*args):
        operands = list(args)
        if partition_name is not None:
            operands.append(bass2jax.partition_id_tensor())
        outs = bass2jax._bass_exec_p.bind(
            *operands,
            out_avals=tuple(out_avals),
            in_names=tuple(all_in_names),
            out_names=tuple(out_names),
            lowering_input_output_aliases=(),
            sim_require_finite=True,
            sim_require_nnan=True,
            nc=nc,
        )
        return tuple(outs)

    n_cores = 8
    devices = jax.devices()[:n_cores]
    mesh = Mesh(np.asarray(devices), ("core",))
    spec = PartitionSpec("core")
    sharded = jax.jit(
        shard_map(_body, mesh=mesh,
                  in_specs=(spec,) * (n_params + len(out_names)),
                  out_specs=(spec,) * len(out_names), check_rep=False))
    sharding = NamedSharding(mesh, spec)
    dev_args = [
        jax.device_put(
            np.concatenate([np.asarray(in_maps[c][nm]) for c in range(n_cores)],
                           axis=0), sharding)
        for nm in in_names
    ] + [
        jax.device_put(
            np.zeros((n_cores * z.shape[0], *z.shape[1:]), z.dtype), sharding)
        for z in zero_outs
    ]

    jax.block_until_ready(sharded(*dev_args))

    def run_batch(n):
        t0 = time.perf_counter()
        outs = None
        for _ in range(n):
            outs = sharded(*dev_args)
        jax.block_until_ready(outs)
        return time.perf_counter() - t0

    best = None
    for _ in range(repeats):
        ta = run_batch(n1)
        tb = run_batch(n2)
        marg = (tb - ta) / (n2 - n1)
        best = marg if best is None else min(best, marg)
    return best * 1e9
